# revision 1
# baseline (speedup 1.0000x reference)
"""CRNN (conv3x3 -> ReLU -> freq-maxpool -> GRU scan -> FC) on 8 Trainium2
NeuronCores, data-parallel over batch (8 items per core).

Structure per core:
  - conv: banded-weight matmuls over the frequency contraction; time shifts
    via column offsets into a padded fp32r tile; two accumulating matmuls per
    f-pair give PSUM [128 = 2f x 64c, 512t]; running tensor_max over f-pairs
    + ReLU(+bias) writes feat[c, t] batch-interleaved into bigU[64:128].
  - xn = W_ihn @ feat + b_ihn precomputed (PE), packed into bigH[64:128].
  - GRU scan with u/v decomposition: h_{k+1} = u_k + v_k, u_k = z_k*h_k,
    v_k = (1-z_k)*n_k. The rz matmul takes [u; feat] (K=128) plus a separate
    v matmul (K=64), so the only late operand on the serial chain is v.
  - FC from bigH h-history, output DMA'd straight from PSUM.
  - The time-half-1 conv work, the second half of xn, and the FC tiles are
    emitted interleaved with the scan steps so they execute in the scan's
    idle engine slots.
"""

import contextlib
import numpy as np

import concourse.bass as bass
import concourse.mybir as mybir
import concourse.tile as tile
from concourse import bacc
from concourse.bass_utils import run_bass_kernel_spmd

F32 = mybir.dt.float32
F32R = mybir.dt.float32r
AF = mybir.ActivationFunctionType
OP = mybir.AluOpType

B, F, T = 64, 64, 1024
C = 64
H = 64
OUT = 2
NCORES = 8
NB = B // NCORES
NFP = F // 2


def build_crnn(nb=NB, t_steps=T, reps=1, phases=("conv", "xn", "scan", "fc"),
               interleave=True):
    nc = bacc.Bacc("TRN2", target_bir_lowering=False, debug=False)
    TB = t_steps * nb
    NTH = max(1, t_steps // 512)
    THW = min(512, t_steps)
    NJ = max(1, TB // 512)
    JW = min(512, TB)
    full = len(phases) == 4
    inter = interleave and full and t_steps == T

    x_d = nc.declare_dram_parameter("x", [nb, F, t_steps], F32, isOutput=False)
    convA_d = nc.declare_dram_parameter("convA", [128, NFP * 128], F32, isOutput=False)
    convB_d = nc.declare_dram_parameter("convB", [64, NFP * 128], F32, isOutput=False)
    cb_d = nc.declare_dram_parameter("conv_bias", [C, 1], F32, isOutput=False)
    wrz_d = nc.declare_dram_parameter("w_rz_lhsT", [128, 128], F32, isOutput=False)
    wn_d = nc.declare_dram_parameter("w_n_lhsT", [H, H], F32, isOutput=False)
    win_d = nc.declare_dram_parameter("w_in_lhsT", [C, H], F32, isOutput=False)
    brz_d = nc.declare_dram_parameter("b_rz", [128, 1], F32, isOutput=False)
    brzn_d = nc.declare_dram_parameter("b_rz_neg", [H, 1], F32, isOutput=False)
    bhn_d = nc.declare_dram_parameter("b_hn", [H, 1], F32, isOutput=False)
    bin_d = nc.declare_dram_parameter("b_in_row", [1, H], F32, isOutput=False)
    fcw_d = nc.declare_dram_parameter("fc_lhsT", [H, OUT], F32, isOutput=False)
    fcb_d = nc.declare_dram_parameter("fc_b_row", [1, OUT], F32, isOutput=False)
    out_d = nc.declare_dram_parameter("out", [nb, OUT, t_steps], F32, isOutput=True)

    with tile.TileContext(nc) as tc:
        with (
            tc.tile_pool(name="persist", bufs=1) as persist,
            tc.tile_pool(name="stage", bufs=2) as stage,
            tc.tile_pool(name="x2pool", bufs=1) as x2p,
            tc.tile_pool(name="work", bufs=2) as work,
            tc.tile_pool(name="scanw", bufs=3) as scanw,
            tc.tile_pool(name="pp_conv", bufs=2, space="PSUM") as ppc,
            tc.tile_pool(name="pp_scan", bufs=2, space="PSUM") as pps,
            tc.tile_pool(name="pp_misc", bufs=2, space="PSUM") as ppm,
        ):
            convA = persist.tile([128, NFP * 128], F32R)
            convB = persist.tile([64, NFP * 128], F32R)
            cb = persist.tile([C, 1], F32)
            w_rz = persist.tile([128, 128], F32)
            w_n = persist.tile([H, H], F32)
            w_in_full = persist.tile([128, H], F32)
            w_in = w_in_full[64:128, :]
            b_rz = persist.tile([128, 1], F32)
            b_rz_neg = persist.tile([H, 1], F32)
            b_hn = persist.tile([H, 1], F32)
            b_in = persist.tile([1, H], F32)
            fc_w = persist.tile([H, OUT], F32)
            fc_b = persist.tile([1, OUT], F32)
            ones = persist.tile([1, JW], F32)
            # bigU: rows 0:64 = u_{k-1} at blk k, rows 64:128 = feat_k at blk k
            bigU = persist.tile([128, (t_steps + 1) * nb], F32)
            # bigH: rows 0:64 = h_k at blk k, rows 64:128 = xn_k at blk k
            bigH = persist.tile([128, (t_steps + 1) * nb], F32)
            v_zero = persist.tile([H, nb], F32)

            CW = NFP * 128 // 4
            for ci in range(4):
                cs = slice(ci * CW, (ci + 1) * CW)
                stg = stage.tile([128, CW], F32, tag="stg", name="stg")
                nc.sync.dma_start(out=stg, in_=convA_d[:, cs])
                nc.vector.tensor_copy(convA[:, cs], stg)
            for ci in range(4):
                cs = slice(ci * CW, (ci + 1) * CW)
                stg = stage.tile([128, CW], F32, tag="stg", name="stgb")
                nc.sync.dma_start(out=stg[0:64, :], in_=convB_d[:, cs])
                nc.vector.tensor_copy(convB[:, cs], stg[0:64, :])

            nc.sync.dma_start(out=cb, in_=cb_d[:, :])
            nc.sync.dma_start(out=w_rz, in_=wrz_d[:, :])
            nc.sync.dma_start(out=w_n, in_=wn_d[:, :])
            nc.sync.dma_start(out=w_in, in_=win_d[:, :])
            nc.sync.dma_start(out=b_rz, in_=brz_d[:, :])
            nc.sync.dma_start(out=b_rz_neg, in_=brzn_d[:, :])
            nc.sync.dma_start(out=b_hn, in_=bhn_d[:, :])
            nc.sync.dma_start(out=b_in, in_=bin_d[:, :])
            nc.sync.dma_start(out=fc_w, in_=fcw_d[:, :])
            nc.sync.dma_start(out=fc_b, in_=fcb_d[:, :])
            nc.vector.memset(ones, 1.0)
            nc.vector.memset(bigU[0:64, 0:nb], 0.0)   # u_{-1} = 0
            nc.vector.memset(bigH[0:64, 0:nb], 0.0)   # h_0 = 0
            nc.vector.memset(v_zero, 0.0)             # v_{-1} = 0
            if not full:
                nc.vector.memset(bigU[:, :], 0.0)
                nc.vector.memset(bigH[:, :], 0.0)

            # ---------- X2R staging (persistent, per batch) ----------
            X2Rs = []
            if "conv" in phases:
                for b in range(nb):
                    X2 = x2p.tile([128, t_steps + 2], F32, tag="x2", name="x2")
                    nc.sync.dma_start(out=X2[0:64, 1 : t_steps + 1], in_=x_d[b, :, :])
                    nc.sync.dma_start(out=X2[64:128, 0:t_steps], in_=x_d[b, :, :])
                    nc.vector.memset(X2[0:64, 0:1], 0.0)
                    nc.vector.memset(X2[0:64, t_steps + 1 : t_steps + 2], 0.0)
                    nc.vector.memset(X2[64:128, t_steps : t_steps + 2], 0.0)
                    X2R = persist.tile([128, t_steps + 2], F32R, name=f"x2r{b}")
                    nc.vector.tensor_copy(X2R, X2)
                    X2Rs.append(X2R)

            # ---------- emission units ----------
            conv_state = {}

            def conv_mm(b, th, fp):
                ps = ppc.tile([128, THW], F32, tag="cps", name="cps")
                X2R = X2Rs[b]
                nc.tensor.matmul(
                    ps, convA[:, fp * 128 : (fp + 1) * 128],
                    X2R[:, th * THW : th * THW + THW],
                    start=True, stop=False,
                )
                nc.tensor.matmul(
                    ps, convB[:, fp * 128 : (fp + 1) * 128],
                    X2R[0:64, th * THW + 2 : th * THW + THW + 2],
                    start=False, stop=True,
                )
                if fp == 0:
                    macc = work.tile([128, THW], F32, tag="macc", name="macc")
                    conv_state[(b, th)] = macc
                    nc.vector.tensor_copy(macc, ps)
                else:
                    nc.vector.tensor_max(conv_state[(b, th)],
                                         conv_state[(b, th)], ps)

            def conv_tail(b, th):
                macc = conv_state.pop((b, th))
                mhi = work.tile([64, THW], F32, tag="mhi", name="mhi")
                nc.vector.tensor_copy(mhi, macc[64:128, :])
                m2 = work.tile([64, THW], F32, tag="m2", name="m2")
                nc.vector.tensor_max(m2, macc[0:64, :], mhi)
                out_ap = bigU[64:128, th * THW * nb + b : (th * THW + THW) * nb : nb]
                nc.scalar.activation(out_ap, m2, AF.Relu, bias=cb)

            def xn_unit(j):
                ps = ppm.tile([H, JW], F32, tag="mps", name="xnps")
                nc.tensor.matmul(
                    ps, w_in, bigU[64:128, j * JW : (j + 1) * JW],
                    start=True, stop=False,
                )
                nc.tensor.matmul(ps, b_in, ones, start=False, stop=True)
                nc.scalar.copy(bigH[64:128, j * JW : (j + 1) * JW], ps)

            def fc_unit(j):
                ps = ppm.tile([OUT, JW], F32, tag="mps", name="fcps")
                nc.tensor.matmul(
                    ps, fc_w, bigH[0:64, nb + j * JW : nb + (j + 1) * JW],
                    start=True, stop=False,
                )
                nc.tensor.matmul(ps, fc_b, ones, start=False, stop=True)
                ob = work.tile([OUT, JW], F32, tag="ob", name="ob")
                nc.scalar.copy(ob, ps)
                tpj = JW // nb
                for b in range(nb):
                    nc.sync.dma_start(
                        out=out_d[b, 0:OUT, j * tpj : (j + 1) * tpj],
                        in_=ob[:, b : JW : nb],
                    )

            def scan_step(k, prev_v):
                col = slice(k * nb, (k + 1) * nb)
                ncol = slice((k + 1) * nb, (k + 2) * nb)
                psum_rz = pps.tile([128, nb], F32, tag="rz", name="rz")
                psum_hn = pps.tile([H, nb], F32, tag="hn", name="hn")
                nc.tensor.matmul(psum_rz, w_rz, bigU[:, col], start=True, stop=False)
                nc.tensor.matmul(psum_hn, w_n, bigH[0:64, col], start=True, stop=True)
                nc.tensor.matmul(psum_rz, w_rz[0:64, :], prev_v, start=False, stop=True)

                r_s = scanw.tile([H, nb], F32, tag="rs", name="rs")
                nc.scalar.activation(r_s, psum_rz[0:64, :], AF.Sigmoid,
                                     bias=b_rz[0:64, :])
                z_s = scanw.tile([H, nb], F32, tag="zs", name="zs")
                nc.scalar.activation(z_s, psum_rz[64:128, :], AF.Sigmoid,
                                     bias=b_rz[64:128, :])
                zb_s = scanw.tile([H, nb], F32, tag="zbs", name="zbs")
                nc.scalar.activation(zb_s, psum_rz[64:128, :], AF.Sigmoid,
                                     bias=b_rz_neg, scale=-1.0)
                nc.vector.tensor_mul(bigU[0:64, ncol], z_s, bigH[0:64, col])
                q = scanw.tile([128, nb], F32, tag="q", name="q")
                nc.vector.scalar_tensor_tensor(
                    out=q[64:128, :], in0=psum_hn, scalar=b_hn, in1=r_s,
                    op0=OP.add, op1=OP.mult,
                )
                q2 = scanw.tile([H, nb], F32, tag="q2", name="q2")
                nc.vector.tensor_add(q2, q[64:128, :], bigH[64:128, col])
                n_t = scanw.tile([H, nb], F32, tag="n", name="n")
                nc.scalar.activation(n_t, q2, AF.Tanh)
                v_t = scanw.tile([H, nb], F32, tag="v", name="v")
                nc.vector.tensor_mul(v_t, zb_s, n_t)
                nc.vector.tensor_add(bigH[0:64, ncol], bigU[0:64, ncol], v_t)
                return v_t

            rep_ctx = tc.For_i(0, reps, 1) if reps > 1 else contextlib.nullcontext()
            with rep_ctx:
                if not inter:
                    for b in range(nb if "conv" in phases else 0):
                        for th in range(NTH):
                            for fp in range(NFP):
                                conv_mm(b, th, fp)
                            conv_tail(b, th)
                    for j in range(NJ if "xn" in phases else 0):
                        xn_unit(j)
                    prev_v = v_zero
                    for k in range(t_steps if "scan" in phases else 0):
                        prev_v = scan_step(k, prev_v)
                    for j in range(NJ if "fc" in phases else 0):
                        fc_unit(j)
                else:
                    # th=0 conv upfront + first-half xn
                    for b in range(nb):
                        for fp in range(NFP):
                            conv_mm(b, 0, fp)
                        conv_tail(b, 0)
                    for j in range(NJ // 2):
                        xn_unit(j)

                    # conv th=1 spread over scan steps [8, 440); 2nd-half xn
                    # after it; each fc tile as soon as its h-range is done.
                    units = []
                    for b in range(nb):
                        for fp in range(NFP):
                            units.append(("mm", b, fp))
                        units.append(("tail", b))
                    sched = {}
                    lo, hi = 8, 440
                    for i, u in enumerate(units):
                        k_at = lo + (i * (hi - lo)) // len(units)
                        sched.setdefault(k_at, []).append(u)
                    for j in range(NJ // 2, NJ):
                        sched.setdefault(444 + 8 * (j - NJ // 2), []).append(("xn", j))
                    tpj = JW // nb
                    for j in range(NJ):
                        k_at = (j + 1) * tpj
                        if k_at < t_steps:
                            sched.setdefault(k_at, []).append(("fc", j))

                    prev_v = v_zero
                    for k in range(t_steps):
                        prev_v = scan_step(k, prev_v)
                        for u in sched.get(k, ()):
                            if u[0] == "mm":
                                conv_mm(u[1], 1, u[2])
                            elif u[0] == "tail":
                                conv_tail(u[1], 1)
                            elif u[0] == "xn":
                                xn_unit(u[1])
                            elif u[0] == "fc":
                                fc_unit(u[1])
                    for j in range(NJ):
                        if (j + 1) * tpj >= t_steps:
                            fc_unit(j)

    nc.finalize()
    return nc


def prep_weights(conv_w, conv_b, w_ih, w_hh, b_ih, b_hh, fc_w, fc_b):
    """Host-side rearrangement of the small weights into device layouts."""
    conv_w = np.asarray(conv_w, np.float32)
    A = np.zeros((128, NFP * 128), np.float32)
    Bm = np.zeros((64, NFP * 128), np.float32)
    for fp in range(NFP):
        for fo in range(2):
            fout = 2 * fp + fo
            for fprime in range(max(0, fout - 1), min(64, fout + 2)):
                i = fprime - fout + 1
                cols = slice(fp * 128 + fo * 64, fp * 128 + fo * 64 + 64)
                A[fprime, cols] = conv_w[:, 0, i, 0]
                A[64 + fprime, cols] = conv_w[:, 0, i, 1]
                Bm[fprime, cols] = conv_w[:, 0, i, 2]
    w_ih = np.asarray(w_ih, np.float32)
    w_hh = np.asarray(w_hh, np.float32)
    b_ih = np.asarray(b_ih, np.float32)
    b_hh = np.asarray(b_hh, np.float32)
    return {
        "convA": A,
        "convB": Bm,
        "conv_bias": np.asarray(conv_b, np.float32).reshape(C, 1),
        "w_rz_lhsT": np.concatenate(
            [w_hh[0:128, :].T, w_ih[0:128, :].T], axis=0
        ).astype(np.float32).copy(),
        "w_n_lhsT": w_hh[128:192, :].T.astype(np.float32).copy(),
        "w_in_lhsT": w_ih[128:192, :].T.astype(np.float32).copy(),
        "b_rz": (b_ih[0:128] + b_hh[0:128]).reshape(128, 1).astype(np.float32),
        "b_rz_neg": (-(b_ih[64:128] + b_hh[64:128])).reshape(H, 1).astype(np.float32),
        "b_hn": b_hh[128:192].reshape(H, 1).astype(np.float32),
        "b_in_row": b_ih[128:192].reshape(1, H).astype(np.float32),
        "fc_lhsT": np.asarray(fc_w, np.float32).T.copy(),
        "fc_b_row": np.asarray(fc_b, np.float32).reshape(1, OUT),
    }


_NC_CACHE = {}


def _get_nc():
    if "nc" not in _NC_CACHE:
        _NC_CACHE["nc"] = build_crnn()
    return _NC_CACHE["nc"]


def run(inputs, trace=False):
    """Returns (out [B, OUT, T], BassKernelResults)."""
    x = np.asarray(inputs["x"], np.float32)
    wd = prep_weights(
        inputs["conv_w"], inputs["conv_b"], inputs["w_ih"], inputs["w_hh"],
        inputs["b_ih"], inputs["b_hh"], inputs["fc_w"], inputs["fc_b"],
    )
    nc = _get_nc()
    in_maps = []
    for i in range(NCORES):
        m = dict(wd)
        m["x"] = np.ascontiguousarray(x[i * NB : (i + 1) * NB])
        in_maps.append(m)
    res = run_bass_kernel_spmd(nc, in_maps, list(range(NCORES)), trace=trace)
    out = np.concatenate([res.results[i]["out"] for i in range(NCORES)], axis=0)
    return out, res


def kernel(**inputs) -> np.ndarray:
    out, _ = run(inputs, trace=False)
    return out



# revision 15
# speedup vs baseline: 1.9576x; 1.9576x over previous
"""CRNN (conv3x3 -> ReLU -> freq-maxpool -> GRU -> FC) on 8 Trainium2
NeuronCores, data-parallel over batch (8 items per core).

Key idea vs a serial GRU scan: the GRU recurrence
    h_t = z_t*h_{t-1} + (1-z_t)*n_t
is solved by Picard (fixed-point) iteration: gates z/r/n are computed from
the PREVIOUS h estimate with wide batched matmuls over all T at once, then
the linear recurrence is solved exactly by the DVE tensor_tensor_scan
instruction (1024 steps in one instruction per batch item). Weights are
~0.1-scale so the iteration contracts ~0.3x/iter; NIT=8 reaches ~1e-4
relative error (tolerance 2e-2), with bf16 noise floor ~5e-3.

Everything wide runs in bf16 (PE matmuls 1 cy/col, DVE 4x mode); PSUM and
the scan state stay fp32. Layout is b-major with a guard column per batch:
col(b, t) = b*1025 + 1 + t; the guard col (t=-1) holds h=0 so the shifted
h_{t-1} read is a plain AP offset.

Conv: banded-weight bf16 matmuls (as before) with the freq-max reduction
split across Act (relu-first then bf16 4x max), DVE, and Pool engines.
"""

import contextlib
import numpy as np
import ml_dtypes

import concourse.bass as bass
import concourse.mybir as mybir
import concourse.tile as tile
from concourse import bacc
from concourse.bass_utils import run_bass_kernel_spmd

F32 = mybir.dt.float32
F32R = mybir.dt.float32r
BF16 = mybir.dt.bfloat16
AF = mybir.ActivationFunctionType
OP = mybir.AluOpType

B, F, T = 64, 64, 1024
C = 64
H = 64
OUT = 2
NCORES = 8
NB = B // NCORES          # 8 batch items per core
NFP = F // 2              # 32 f-pair blocks
SEG = T + 1               # cols per batch incl. guard col at t=-1
W = NB * SEG              # 8200
NIT = 8                   # Picard iterations
CH = 512                  # column chunk (one PSUM bank)

# conv f-pair routing (Pool cannot read PSUM, so it only gets bf16
# residual maxes from the Act-relu-first stream)
CONV_ACT = 18   # relu-first on Act -> bf16, residual max on DVE (4x)
CONV_DVE = 14   # direct fp32 running max on DVE (reads PSUM)
CONV_POOL_RESID = 0  # Pool supports no tensor-max, no PSUM reads


def build_crnn(nit=NIT, reps=1, parts=("conv", "g", "picard", "fc")):
    nc = bacc.Bacc("TRN2", target_bir_lowering=False, debug=False)

    x_d = nc.declare_dram_parameter("x", [NB, F, T], BF16, isOutput=False)
    convA_d = nc.declare_dram_parameter("convA", [128, NFP * 128], BF16, isOutput=False)
    convB_d = nc.declare_dram_parameter("convB", [64, NFP * 128], BF16, isOutput=False)
    cb2_d = nc.declare_dram_parameter("conv_bias2", [128, 1], F32, isOutput=False)
    wzr_d = nc.declare_dram_parameter("w_zr_lhsT", [H, 128], F32, isOutput=False)
    wihzr_d = nc.declare_dram_parameter("w_ihzr_lhsT", [C, 128], F32, isOutput=False)
    wn_d = nc.declare_dram_parameter("w_n_lhsT", [H, H], F32, isOutput=False)
    wihn_d = nc.declare_dram_parameter("w_ihn_lhsT", [C, H], F32, isOutput=False)
    bzr_d = nc.declare_dram_parameter("b_zr", [128, 1], F32, isOutput=False)
    bhn_d = nc.declare_dram_parameter("b_hn_up", [128, 1], F32, isOutput=False)
    bihn_d = nc.declare_dram_parameter("b_ihn", [H, 1], F32, isOutput=False)
    id_d = nc.declare_dram_parameter("ident", [128, 128], F32, isOutput=False)
    fcw_d = nc.declare_dram_parameter("fc_lhsT", [H, OUT], F32, isOutput=False)
    fcb_d = nc.declare_dram_parameter("fc_b_row", [1, OUT], F32, isOutput=False)
    out_d = nc.declare_dram_parameter("out", [NB, OUT, T], F32, isOutput=True)

    with tile.TileContext(nc) as tc:
        with (
            tc.tile_pool(name="persist", bufs=1) as persist,
            tc.tile_pool(name="work", bufs=3) as work,
            tc.tile_pool(name="sw", bufs=4) as sw,
            tc.tile_pool(name="ppbig", bufs=3, space="PSUM") as ppbig,
            tc.tile_pool(name="pp64", bufs=3, space="PSUM") as pp64,
        ):
            convA = persist.tile([128, NFP * 128], BF16)
            convB = persist.tile([64, NFP * 128], BF16)
            cb2 = persist.tile([128, 1], F32)
            b_zr = persist.tile([128, 1], F32)
            b_hn = persist.tile([128, 1], F32)
            b_ihn = persist.tile([H, 1], F32)
            for t, d in ((convA, convA_d), (convB, convB_d), (cb2, cb2_d),
                         (b_zr, bzr_d), (b_hn, bhn_d), (b_ihn, bihn_d)):
                nc.sync.dma_start(out=t, in_=d[:, :])

            # small GRU/FC weights: DMA f32 then convert to f32r (exact fp32
            # matmul operands at 1 cy/col)
            w_zr = persist.tile([H, 128], F32R)
            w_ihzr = persist.tile([C, 128], F32R)
            w_n = persist.tile([H, H], F32R)
            w_ihn = persist.tile([C, H], F32R)
            ident = persist.tile([128, 128], F32R)
            fc_w = persist.tile([H, OUT], F32R)
            fc_br = persist.tile([1, OUT], F32R)
            onesR = persist.tile([1, CH], F32R)
            for t, d in ((w_zr, wzr_d), (w_ihzr, wihzr_d), (w_n, wn_d),
                         (w_ihn, wihn_d), (ident, id_d), (fc_w, fcw_d),
                         (fc_br, fcb_d)):
                stg = sw.tile(list(t.shape), F32, tag="wstg", name="wstg")
                nc.sync.dma_start(out=stg, in_=d[:, :])
                nc.vector.tensor_copy(t, stg)
            ones_f = sw.tile([1, CH], F32, tag="wstg", name="ones")
            nc.vector.memset(ones_f, 1.0)
            nc.vector.tensor_copy(onesR, ones_f)

            Gzr = persist.tile([128, W], F32R)
            Gn = persist.tile([C, W], F32)
            # zr/d live in 2-batch rings (the scan consumes them per batch)
            zrS = persist.tile([128, 2 * SEG], BF16)
            dS = persist.tile([C, 2 * SEG], F32)
            hS = persist.tile([H, W], F32R)
            zstg = sw.tile([H, CH], F32, tag="wstg2", name="zstg")
            nc.vector.memset(zstg, 0.0)
            for ci in range(W // CH):
                nc.vector.tensor_copy(hS[:, ci * CH:(ci + 1) * CH], zstg)
            nc.vector.tensor_copy(hS[:, W - (W % CH):W], zstg[:, 0:W % CH])

            X2s = []
            if "conv" in parts:
                for b in range(NB):
                    X2 = persist.tile([128, T + 2], BF16)
                    nc.sync.dma_start(out=X2[0:64, 1:T + 1], in_=x_d[b, :, :])
                    nc.sync.dma_start(out=X2[64:128, 0:T], in_=x_d[b, :, :])
                    nc.vector.memset(X2[0:64, 0:1], 0.0)
                    nc.vector.memset(X2[0:64, T + 1:T + 2], 0.0)
                    nc.vector.memset(X2[64:128, T:T + 2], 0.0)
                    X2s.append(X2)

            # fp -> route for the freq-max reduction: "A" = Act relu-first
            # (residual bf16 max on DVE, every few on Pool), "D" = DVE direct.
            # Interleave so the chains run concurrently.
            routes = ["A"] * CONV_ACT + ["D"] * CONV_DVE
            order = sorted(range(NFP), key=lambda i: (i * 7) % NFP)
            fp_route = {fp: routes[i] for i, fp in enumerate(order)}

            def conv_chunk(b, th):
                X2 = X2s[b]
                c0 = b * SEG + 1 + th * CH
                accD = accA = accP = None
                na = 0
                for fp in range(NFP):
                    ps = ppbig.tile([128, CH], F32, tag="bigps", name="cps")
                    nc.tensor.matmul(
                        ps, convA[:, fp * 128:(fp + 1) * 128],
                        X2[:, th * CH: th * CH + CH], start=True, stop=False)
                    nc.tensor.matmul(
                        ps, convB[:, fp * 128:(fp + 1) * 128],
                        X2[0:64, th * CH + 2: th * CH + CH + 2],
                        start=False, stop=True)
                    if fp_route[fp] == "A":
                        na += 1
                        if accA is None:
                            accA = work.tile([128, CH], BF16, tag="accA", name="accA")
                            nc.scalar.activation(accA, ps, AF.Relu, bias=cb2)
                        elif na <= CONV_POOL_RESID + 1 and accP is None:
                            accP = work.tile([128, CH], BF16, tag="accP", name="accP")
                            nc.scalar.activation(accP, ps, AF.Relu, bias=cb2)
                        else:
                            tmp = sw.tile([128, CH], BF16, tag="atmp", name="atmp")
                            nc.scalar.activation(tmp, ps, AF.Relu, bias=cb2)
                            if na <= CONV_POOL_RESID + 1:
                                nc.gpsimd.tensor_max(accP, accP, tmp)
                            else:
                                nc.vector.tensor_max(accA, accA, tmp)
                    else:
                        if accD is None:
                            accD = work.tile([128, CH], F32, tag="accD", name="accD")
                            nc.vector.tensor_copy(accD, ps)
                        else:
                            nc.vector.tensor_max(accD, accD, ps)
                # combine the three accumulators
                mr = work.tile([128, CH], BF16, tag="mr", name="mr")
                nc.scalar.activation(mr, accD, AF.Relu, bias=cb2)
                if accP is not None:
                    nc.vector.tensor_max(accA, accA, accP)
                nc.vector.tensor_max(mr, mr, accA)
                mhi = work.tile([64, CH], BF16, tag="mhi", name="mhi")
                nc.vector.tensor_copy(mhi, mr[64:128, :])
                featc = work.tile([64, CH], BF16, tag="featc", name="featc")
                nc.vector.tensor_max(featc, mr[0:64, :], mhi)
                featR = work.tile([64, CH], F32R, tag="featR", name="featR")
                nc.vector.tensor_copy(featR, featc)
                return featR

            def g_chunk(featR, b, th):
                c0 = b * SEG + 1 + th * CH
                cols = slice(c0, c0 + CH)
                psg = ppbig.tile([128, CH], F32, tag="bigps", name="gps")
                nc.tensor.matmul(psg, w_ihzr, featR, start=True, stop=True)
                nc.scalar.copy(Gzr[:, cols], psg)
                psn = pp64.tile([64, CH], F32, tag="ps64", name="gnps")
                nc.tensor.matmul(psn, w_ihn, featR, start=True, stop=True)
                nc.scalar.copy(Gn[:, cols], psn)

            def picard_chunk(b, th):
                c0 = b * SEG + 1 + th * CH
                cols = slice(c0, c0 + CH)
                pcols = slice(c0 - 1, c0 - 1 + CH)
                r0 = (b % 2) * SEG + 1 + th * CH
                rcols = slice(r0, r0 + CH)
                ps_zr = ppbig.tile([128, CH], F32, tag="bigps", name="zrps")
                nc.tensor.matmul(ps_zr, w_zr, hS[:, pcols], start=True, stop=False)
                nc.tensor.matmul(ps_zr, ident, Gzr[:, cols], start=False, stop=True)
                ps_hn = pp64.tile([64, CH], F32, tag="ps64", name="hnps")
                nc.tensor.matmul(ps_hn, w_n, hS[:, pcols], start=True, stop=True)
                nc.scalar.activation(zrS[:, rcols], ps_zr, AF.Sigmoid, bias=b_zr)
                q = sw.tile([64, CH], BF16, tag="q", name="q")
                nc.vector.scalar_tensor_tensor(
                    out=q, in0=ps_hn, scalar=b_hn[64:128, :],
                    in1=zrS[64:128, rcols], op0=OP.add, op1=OP.mult)
                q2 = sw.tile([64, CH], BF16, tag="q2", name="q2")
                nc.gpsimd.tensor_add(q2, q, Gn[:, cols])
                nt = sw.tile([64, CH], BF16, tag="nt", name="nt")
                nc.scalar.activation(nt, q2, AF.Tanh, bias=b_ihn)
                zn = sw.tile([64, CH], BF16, tag="zn", name="zn")
                nc.vector.tensor_mul(zn, zrS[0:64, rcols], nt)
                nc.vector.tensor_sub(dS[:, rcols], nt, zn)

            def fc_chunk(b, th):
                c0 = b * SEG + 1 + th * CH
                cols = slice(c0, c0 + CH)
                psf = pp64.tile([OUT, CH], F32, tag="ps64", name="fcps")
                nc.tensor.matmul(psf, fc_w, hS[:, cols], start=True, stop=False)
                nc.tensor.matmul(psf, fc_br, onesR, start=False, stop=True)
                ob = sw.tile([OUT, CH], F32, tag="ob", name="ob")
                nc.scalar.copy(ob, psf)
                nc.sync.dma_start(
                    out=out_d[b, :, th * CH:(th + 1) * CH], in_=ob)

            rep_ctx = tc.For_i(0, reps, 1) if reps > 1 else contextlib.nullcontext()
            with rep_ctx:
                if "conv" in parts:
                    for b in range(NB):
                        for th in range(2):
                            featR = conv_chunk(b, th)
                            if "g" in parts:
                                g_chunk(featR, b, th)

                if "picard" in parts:
                    for it in range(nit):
                        for b in range(NB):
                            for th in range(2):
                                picard_chunk(b, th)
                            s0 = b * SEG + 1
                            rs0 = (b % 2) * SEG + 1
                            nc.vector.tensor_tensor_scan(
                                out=hS[:, s0:s0 + T],
                                data0=zrS[0:64, rs0:rs0 + T],
                                data1=dS[:, rs0:rs0 + T],
                                initial=0.0, op0=OP.mult, op1=OP.add)

                if "fc" in parts:
                    for b in range(NB):
                        for th in range(2):
                            fc_chunk(b, th)

    nc.finalize()
    return nc


def prep_weights(conv_w, conv_b, w_ih, w_hh, b_ih, b_hh, fc_w, fc_b):
    """Host-side rearrangement of the small weights into device layouts."""
    bf16 = ml_dtypes.bfloat16
    conv_w = np.asarray(conv_w, np.float32)
    A = np.zeros((128, NFP * 128), np.float32)
    Bm = np.zeros((64, NFP * 128), np.float32)
    for fp in range(NFP):
        for fo in range(2):
            fout = 2 * fp + fo
            for fprime in range(max(0, fout - 1), min(64, fout + 2)):
                i = fprime - fout + 1
                cols = slice(fp * 128 + fo * 64, fp * 128 + fo * 64 + 64)
                A[fprime, cols] = conv_w[:, 0, i, 0]
                A[64 + fprime, cols] = conv_w[:, 0, i, 1]
                Bm[fprime, cols] = conv_w[:, 0, i, 2]
    w_ih = np.asarray(w_ih, np.float32)
    w_hh = np.asarray(w_hh, np.float32)
    b_ih = np.asarray(b_ih, np.float32)
    b_hh = np.asarray(b_hh, np.float32)
    conv_b = np.asarray(conv_b, np.float32)
    # gate order on chip is (z; r) so z lands at base partition 0
    b_zr = np.concatenate([b_ih[64:128] + b_hh[64:128], b_ih[0:64] + b_hh[0:64]])
    b_hn_up = np.concatenate([np.zeros(64, np.float32), b_hh[128:192]])
    return {
        "convA": A.astype(bf16),
        "convB": Bm.astype(bf16),
        "conv_bias2": np.tile(conv_b, 2).reshape(128, 1).astype(np.float32),
        "w_zr_lhsT": np.ascontiguousarray(
            np.concatenate([w_hh[64:128].T, w_hh[0:64].T], axis=1)),
        "w_ihzr_lhsT": np.ascontiguousarray(
            np.concatenate([w_ih[64:128].T, w_ih[0:64].T], axis=1)),
        "w_n_lhsT": np.ascontiguousarray(w_hh[128:192].T),
        "w_ihn_lhsT": np.ascontiguousarray(w_ih[128:192].T),
        "b_zr": b_zr.reshape(128, 1).astype(np.float32),
        "b_hn_up": b_hn_up.reshape(128, 1).astype(np.float32),
        "b_ihn": b_ih[128:192].reshape(H, 1).astype(np.float32),
        "ident": np.eye(128, dtype=np.float32),
        "fc_lhsT": np.ascontiguousarray(np.asarray(fc_w, np.float32).T),
        "fc_b_row": np.asarray(fc_b, np.float32).reshape(1, OUT),
    }


def prep_x(x):
    """Full (B, F, T) fp32 -> per-core list of (NB, F, T) bf16 arrays."""
    xb = np.asarray(x, np.float32).astype(ml_dtypes.bfloat16)
    return [np.ascontiguousarray(xb[i * NB:(i + 1) * NB]) for i in range(NCORES)]


_NC_CACHE = {}


def _get_nc():
    if "nc" not in _NC_CACHE:
        _NC_CACHE["nc"] = build_crnn()
    return _NC_CACHE["nc"]


def _build_runner(nc, n_cores):
    """Build the sharded PJRT executable ONCE; reuse across kernel() calls."""
    import jax
    from jax.sharding import Mesh, PartitionSpec
    from jax.experimental.shard_map import shard_map
    from concourse.bass2jax import (
        _bass_exec_p, install_neuronx_cc_hook, partition_id_tensor,
    )

    install_neuronx_cc_hook()
    partition_name = nc.partition_id_tensor.name if nc.partition_id_tensor else None

    in_names, out_names, out_avals, out_shapes = [], [], [], []
    for alloc in nc.m.functions[0].allocations:
        if not isinstance(alloc, mybir.MemoryLocationSet):
            continue
        name = alloc.memorylocations[0].name
        if alloc.kind == "ExternalInput":
            if name != partition_name:
                in_names.append(name)
        elif alloc.kind == "ExternalOutput":
            out_names.append(name)
            shape = tuple(alloc.tensor_shape)
            dtype = mybir.dt.np(alloc.dtype)
            out_avals.append(jax.core.ShapedArray(shape, dtype))
            out_shapes.append((shape, dtype))
    n_params = len(in_names)
    n_outs = len(out_avals)
    all_in_names = list(in_names) + list(out_names)
    if partition_name is not None:
        all_in_names.append(partition_name)
    donate = tuple(range(n_params, n_params + n_outs))

    def _body(*args):
        operands = list(args)
        if partition_name is not None:
            operands.append(partition_id_tensor())
        outs = _bass_exec_p.bind(
            *operands,
            out_avals=tuple(out_avals),
            in_names=tuple(all_in_names),
            out_names=tuple(out_names),
            lowering_input_output_aliases=(),
            sim_require_finite=True,
            sim_require_nnan=True,
            nc=nc,
        )
        return tuple(outs)

    devices = jax.devices()[:n_cores]
    mesh = Mesh(np.asarray(devices), ("core",))
    in_specs = (PartitionSpec("core"),) * (n_params + n_outs)
    out_specs = (PartitionSpec("core"),) * len(out_names)
    sharded = jax.jit(
        shard_map(_body, mesh=mesh, in_specs=in_specs, out_specs=out_specs,
                  check_rep=False),
        donate_argnums=donate,
        keep_unused=True,
    )

    def run_fn(concat_in_map):
        concat_in = [concat_in_map[name] for name in in_names]
        concat_zeros = [
            np.zeros((n_cores * s[0], *s[1:]), d) for s, d in out_shapes
        ]
        out_arrs = sharded(*concat_in, *concat_zeros)
        return {
            name: np.asarray(out_arrs[i]) for i, name in enumerate(out_names)
        }

    return run_fn


def run(inputs, trace=False):
    """Returns (out [B, OUT, T], BassKernelResults)."""
    wd = prep_weights(
        inputs["conv_w"], inputs["conv_b"], inputs["w_ih"], inputs["w_hh"],
        inputs["b_ih"], inputs["b_hh"], inputs["fc_w"], inputs["fc_b"],
    )
    xs = prep_x(inputs["x"])
    nc = _get_nc()
    in_maps = []
    for i in range(NCORES):
        m = dict(wd)
        m["x"] = xs[i]
        in_maps.append(m)
    res = run_bass_kernel_spmd(nc, in_maps, list(range(NCORES)), trace=trace)
    out = np.concatenate([res.results[i]["out"] for i in range(NCORES)], axis=0)
    return out, res


def kernel(**inputs) -> np.ndarray:
    if "runner" not in _NC_CACHE:
        _NC_CACHE["runner"] = _build_runner(_get_nc(), NCORES)
    run_fn = _NC_CACHE["runner"]

    wd = prep_weights(
        inputs["conv_w"], inputs["conv_b"], inputs["w_ih"], inputs["w_hh"],
        inputs["b_ih"], inputs["b_hh"], inputs["fc_w"], inputs["fc_b"],
    )
    # Concatenated-over-cores layout: x slices concat back to the full x
    # (one cast, no reshuffle); per-core-replicated weights tile along axis 0.
    x = np.ascontiguousarray(
        np.asarray(inputs["x"], np.float32)).astype(ml_dtypes.bfloat16)
    concat_in = {"x": x.reshape(NCORES * NB, F, T)}
    for name, w in wd.items():
        concat_in[name] = np.ascontiguousarray(
            np.broadcast_to(w[None], (NCORES, *w.shape))
        ).reshape(NCORES * w.shape[0], *w.shape[1:])
    outs = run_fn(concat_in)
    out = outs["out"].reshape(B, OUT, T)
    return out
